# revision 14
# baseline (speedup 1.0000x reference)
"""Banded local-linear layer (nn_LocalLinearLayer) on 8 trn2 NeuronCores.

out[b, o, c] = sum_p W[o, p] * xpad[b, c, p] + bias[o],  band p in [o, o+25)
xpad = edge-replicate pad of x along L (first/last 12 rows duplicated).

Strategy (v8):
  - Tensor-parallel over output rows: core c owns out rows [512c, 512c+512)
    for ALL batches/channels -> banded weight is sharded 8-way (~138 KB/core)
    instead of replicated (1.06 MB/core).
  - 5 output tiles per core of M=104 rows (last 96): tile t contracts xpad
    rows [104t, 104t+128) -> ONE K=128 banded weight per tile, streamed as 4
    PSUM-bank matmuls of 512 columns.
  - bf16 operands (1 col/cycle on the PE), fp32 PSUM, fp16 output.
  - bias rides as a 105th bf16 column of the weight tensor (a standalone
    [104 partitions x 20 B] bias DMA ran at 40 GB/s -- descriptor hell).
  - Tile 0's x load is split per 512-col PSUM bank so the first real matmul
    starts ~0.5 us after the first chunk lands and the PE stream is
    continuous from there (helps the HAM clock governor reach 2.4 GHz);
    remaining x tiles load whole, all on the Sync HWDGE ring.
  - Each tile's PSUM is TWO tiles (banks 0-1 / 2-3): Tile serializes
    ScalarE/VectorE reads of one PSUM tile, so lo/hi splits let the
    bias-add copies (ACT lo, DVE hi) run in parallel into a shared fp16
    out tile; one 416 KB output DMA per tile on GpSimd (SWDGE, 362 GB/s).
"""

import sys

for _p in ("/opt/trn_rl_repo",):
    if _p not in sys.path:
        sys.path.insert(0, _p)

import ml_dtypes
import numpy as np

import concourse.bass as bass
import concourse.tile as tile
from concourse import bacc, mybir
from concourse.bass_utils import run_bass_kernel_spmd

L = 4096
WIN = 25
PAD = (WIN - 1) // 2  # 12
PADDED = L + 2 * PAD  # 4120
B = 32
C = 64
NCORES = 8
P = 128
ROWS_PC = L // NCORES  # 512 output rows per core
M = P - (WIN - 1)  # 104 output rows per tile
NT = (ROWS_PC + M - 1) // M  # 5 tiles per core
M_LAST = ROWS_PC - (NT - 1) * M  # 96
NFREE = B * C  # 2048
NB = NFREE // 512  # 4 psum banks per tile
WCOL = M + 1  # 105: weight columns + bias column

F32 = mybir.dt.float32
F16 = mybir.dt.float16
BF16 = mybir.dt.bfloat16
NPBF16 = np.dtype(ml_dtypes.bfloat16)


def _host_weights(W: np.ndarray, b: np.ndarray):
    """Band-extract and shard W/b by output row.

    wk[j, o] = W[o, o+j] is the dense band (j in [0, WIN)).
    Per core: wp[k, t, m] = wk[k-m, o0+m] for 0 <= k-m < WIN (o0 = 512c+104t);
    wp[m, t, M] = b[o0+m] (bias column, bf16).
    """
    o = np.arange(L)
    wk = W[o[:, None], o[:, None] + np.arange(WIN)[None, :]].T  # [WIN, L]
    wps = []
    for c in range(NCORES):
        wp = np.zeros((P, NT, WCOL), np.float32)
        for t in range(NT):
            o0 = c * ROWS_PC + t * M
            mt = min(M, ROWS_PC - t * M)
            for j in range(WIN):
                m = np.arange(0, mt)
                wp[m + j, t, m] = wk[j, o0 + m]
            wp[:mt, t, M] = b[o0 : o0 + mt]
        wps.append(wp.astype(NPBF16))
    return wps


def _host_x(x: np.ndarray):
    """x [B, L, C] f32 -> per-core [128, NT, B*C] bf16 overlapping xpad tiles."""
    xp = np.concatenate([x[:, :PAD], x, x[:, -PAD:]], axis=1)  # [B, PADDED, C]
    xpad = np.zeros((B, NCORES * ROWS_PC + P, C), np.float32)
    xpad[:, :PADDED] = xp
    xh = []
    for c in range(NCORES):
        t = np.empty((P, NT, B, C), np.float32)
        for ti in range(NT):
            r0 = c * ROWS_PC + ti * M
            t[:, ti] = xpad[:, r0 : r0 + P].transpose(1, 0, 2)
        xh.append(
            np.ascontiguousarray(t.reshape(P, NT, NFREE).astype(NPBF16))
        )
    return xh


def _build_nc():
    nc = bacc.Bacc("TRN2", target_bir_lowering=False, debug=False, num_devices=NCORES)
    x_d = nc.dram_tensor("x", [P, NT, NFREE], BF16, kind="ExternalInput").ap()
    wp_d = nc.dram_tensor("wp", [P, NT, WCOL], BF16, kind="ExternalInput").ap()
    out_d = nc.dram_tensor("out", [M, NT, NFREE], F16, kind="ExternalOutput").ap()

    with tile.TileContext(nc) as tc:
        with (
            tc.tile_pool(name="main", bufs=1) as pool,
            tc.tile_pool(name="ps", bufs=2, space=bass.MemorySpace.PSUM) as pspool,
        ):
            x_s = pool.tile([P, NT, NFREE], BF16)
            wp_s = pool.tile([P, NT, WCOL], BF16)
            bias_s = pool.tile([M, NT], F32)
            out_s = pool.tile([M, NT, NFREE], F16)
            scratch = pool.tile([P, 512], BF16)

            # tile-0 weights first (gates the first matmul), then the rest
            nc.scalar.dma_start(wp_s[:, 0:1], wp_d[:, 0:1])
            nc.scalar.dma_start(wp_s[:, 1:NT], wp_d[:, 1:NT])
            # tile 0's x split per psum bank for the earliest possible start
            for F in range(NB):
                s = slice(F * 512, (F + 1) * 512)
                nc.sync.dma_start(x_s[:, 0, s], x_d[:, 0, s])
            for t in range(1, NT):
                nc.sync.dma_start(x_s[:, t : t + 1], x_d[:, t : t + 1])

            # bf16 bias column -> f32 (tensor_scalar needs an f32 operand)
            nc.vector.tensor_scalar_add(
                bias_s[:, 0:1], wp_s[0:M, 0:1, M : M + 1], 0.0
            )
            nc.vector.tensor_scalar_add(
                bias_s[:, 1:NT], wp_s[0:M, 1:NT, M : M + 1], 0.0
            )

            # PE clock warm-up on zeroed scratch, runs during the x0 DMA wait
            nc.vector.memset(scratch[:], 0.0)
            ps_warm = pspool.tile([P, NFREE // 2], F32, tag="pslo")
            for _ in range(3):
                nc.tensor.matmul(
                    ps_warm[:, 0:512], scratch[:, 0:P], scratch[:], start=True, stop=True
                )

            half = NFREE // 2
            for t in range(NT):
                ps_lo = pspool.tile([P, half], F32, tag="pslo")
                ps_hi = pspool.tile([P, half], F32, tag="pshi")
                for F in range(NB):
                    s = slice(F * 512, (F + 1) * 512)
                    dst = ps_lo if F < 2 else ps_hi
                    ds = slice((F % 2) * 512, (F % 2) * 512 + 512)
                    nc.tensor.matmul(
                        dst[0:M, ds],
                        wp_s[:, t, 0:M],
                        x_s[:, t, s],
                        start=True,
                        stop=True,
                    )
                nc.scalar.activation(
                    out_s[:, t, 0:half],
                    ps_lo[0:M],
                    mybir.ActivationFunctionType.Identity,
                    bias=bias_s[:, t : t + 1],
                )
                nc.vector.tensor_scalar_add(
                    out_s[:, t, half:NFREE],
                    ps_hi[0:M],
                    bias_s[:, t : t + 1],
                )
                nc.gpsimd.dma_start(out_d[:, t], out_s[:, t])

    nc.compile()
    return nc


_NC = None


def _get_nc():
    global _NC
    if _NC is None:
        _NC = _build_nc()
    return _NC


def _make_in_maps(x, W, b):
    wps = _host_weights(
        np.asarray(W, dtype=np.float32), np.asarray(b, dtype=np.float32)
    )
    xh = _host_x(np.asarray(x, dtype=np.float32))
    return [{"x": xh[c], "wp": wps[c]} for c in range(NCORES)]


def _gather(results):
    out = np.empty((B, L, C), np.float32)
    for c, r in enumerate(results):
        oh = np.asarray(r["out"]).reshape(M, NT, B, C)  # [104, 5, B, C]
        for t in range(NT):
            mt = min(M, ROWS_PC - t * M)
            r0 = c * ROWS_PC + t * M
            out[:, r0 : r0 + mt] = oh[:mt, t].transpose(1, 0, 2)
    return out


def kernel(x: np.ndarray, W: np.ndarray, b: np.ndarray) -> np.ndarray:
    nc = _get_nc()
    res = run_bass_kernel_spmd(nc, _make_in_maps(x, W, b), list(range(NCORES)))
    return _gather(res.results)


if __name__ == "__main__":
    rng = np.random.default_rng(0)
    x = rng.standard_normal((B, L, C), dtype=np.float32)
    W = rng.standard_normal((L, PADDED), dtype=np.float32) * 0.02
    b = rng.standard_normal((L,), dtype=np.float32) * 0.02
    print(kernel(x, W, b).shape)


# revision 16
# speedup vs baseline: 1.1048x; 1.1048x over previous
"""Banded local-linear layer (nn_LocalLinearLayer) on 8 trn2 NeuronCores.

out[b, o, c] = sum_p W[o, p] * xpad[b, c, p] + bias[o],  band p in [o, o+25)
xpad = edge-replicate pad of x along L (first/last 12 rows duplicated).

Strategy (v8):
  - Tensor-parallel over output rows: core c owns out rows [512c, 512c+512)
    for ALL batches/channels -> banded weight is sharded 8-way (~138 KB/core)
    instead of replicated (1.06 MB/core).
  - 5 output tiles per core of M=104 rows (last 96): tile t contracts xpad
    rows [104t, 104t+128) -> ONE K=128 banded weight per tile, streamed as 4
    PSUM-bank matmuls of 512 columns.
  - bf16 operands (1 col/cycle on the PE), fp32 PSUM, fp16 output.
  - bias rides as a 105th bf16 column of the weight tensor (a standalone
    [104 partitions x 20 B] bias DMA ran at 40 GB/s -- descriptor hell).
  - Tile 0's x load is split per 512-col PSUM bank so the first real matmul
    starts ~0.5 us after the first chunk lands and the PE stream is
    continuous from there (helps the HAM clock governor reach 2.4 GHz);
    remaining x tiles load whole, all on the Sync HWDGE ring.
  - Each tile's PSUM is TWO tiles (banks 0-1 / 2-3): Tile serializes
    ScalarE/VectorE reads of one PSUM tile, so lo/hi splits let the
    bias-add copies (ACT lo, DVE hi) run in parallel into a shared fp16
    out tile; one 416 KB output DMA per tile on GpSimd (SWDGE, 362 GB/s).
"""

import sys

for _p in ("/opt/trn_rl_repo",):
    if _p not in sys.path:
        sys.path.insert(0, _p)

import ml_dtypes
import numpy as np

import concourse.bass as bass
import concourse.tile as tile
from concourse import bacc, mybir
from concourse.bass_utils import run_bass_kernel_spmd

L = 4096
WIN = 25
PAD = (WIN - 1) // 2  # 12
PADDED = L + 2 * PAD  # 4120
B = 32
C = 64
NCORES = 8
P = 128
ROWS_PC = L // NCORES  # 512 output rows per core
M = P - (WIN - 1)  # 104 output rows per tile
NT = (ROWS_PC + M - 1) // M  # 5 tiles per core
M_LAST = ROWS_PC - (NT - 1) * M  # 96
NFREE = B * C  # 2048
NB = NFREE // 512  # 4 psum banks per tile
WCOL = M + 1  # 105: weight columns + bias column

F32 = mybir.dt.float32
F16 = mybir.dt.float16
BF16 = mybir.dt.bfloat16
NPBF16 = np.dtype(ml_dtypes.bfloat16)


def _host_weights(W: np.ndarray, b: np.ndarray):
    """Band-extract and shard W/b by output row.

    wk[j, o] = W[o, o+j] is the dense band (j in [0, WIN)).
    Per core: wp[k, t, m] = wk[k-m, o0+m] for 0 <= k-m < WIN (o0 = 512c+104t);
    wp[m, t, M] = b[o0+m] (bias column, bf16).
    """
    o = np.arange(L)
    wk = W[o[:, None], o[:, None] + np.arange(WIN)[None, :]].T  # [WIN, L]
    wps = []
    for c in range(NCORES):
        wp = np.zeros((P, NT, WCOL), np.float32)
        for t in range(NT):
            o0 = c * ROWS_PC + t * M
            mt = min(M, ROWS_PC - t * M)
            for j in range(WIN):
                m = np.arange(0, mt)
                wp[m + j, t, m] = wk[j, o0 + m]
            wp[:mt, t, M] = b[o0 : o0 + mt]
        wps.append(wp.astype(NPBF16))
    return wps


def _host_x(x: np.ndarray):
    """x [B, L, C] f32 -> per-core [128, NT, B*C] bf16 overlapping xpad tiles."""
    xp = np.concatenate([x[:, :PAD], x, x[:, -PAD:]], axis=1)  # [B, PADDED, C]
    xpad = np.zeros((B, NCORES * ROWS_PC + P, C), np.float32)
    xpad[:, :PADDED] = xp
    xh = []
    for c in range(NCORES):
        t = np.empty((P, NT, B, C), np.float32)
        for ti in range(NT):
            r0 = c * ROWS_PC + ti * M
            t[:, ti] = xpad[:, r0 : r0 + P].transpose(1, 0, 2)
        xh.append(
            np.ascontiguousarray(t.reshape(P, NT, NFREE).astype(NPBF16))
        )
    return xh


def _build_nc():
    nc = bacc.Bacc("TRN2", target_bir_lowering=False, debug=False, num_devices=NCORES)
    x_d = nc.dram_tensor("x", [P, NT, NFREE], BF16, kind="ExternalInput").ap()
    wp_d = nc.dram_tensor("wp", [P, NT, WCOL], BF16, kind="ExternalInput").ap()
    out_d = nc.dram_tensor("out", [M, NT, NFREE], F16, kind="ExternalOutput").ap()

    with tile.TileContext(nc) as tc:
        with (
            tc.tile_pool(name="main", bufs=1) as pool,
            tc.tile_pool(name="ps", bufs=2, space=bass.MemorySpace.PSUM) as pspool,
        ):
            x_s = pool.tile([P, NT, NFREE], BF16)
            wp_s = pool.tile([P, NT, WCOL], BF16)
            bias_s = pool.tile([M, NT], F32)
            out_s = pool.tile([M, NT, NFREE], F16)
            scratch = pool.tile([P, 512], BF16)

            # tile-0 weights first (gates the first matmul), then the rest
            nc.scalar.dma_start(wp_s[:, 0:1], wp_d[:, 0:1])
            nc.scalar.dma_start(wp_s[:, 1:NT], wp_d[:, 1:NT])
            # tile 0's x in two halves for the earliest matmul start; the
            # rest whole-tile (>8 in-flight DMAs hit sem-lane recycling)
            half = NFREE // 2
            nc.sync.dma_start(x_s[:, 0, 0:half], x_d[:, 0, 0:half])
            nc.sync.dma_start(x_s[:, 0, half:NFREE], x_d[:, 0, half:NFREE])
            for t in range(1, NT):
                nc.sync.dma_start(x_s[:, t : t + 1], x_d[:, t : t + 1])

            # bf16 bias column -> f32 (tensor_scalar needs an f32 operand)
            nc.vector.tensor_scalar_add(
                bias_s[:, 0:1], wp_s[0:M, 0:1, M : M + 1], 0.0
            )
            nc.vector.tensor_scalar_add(
                bias_s[:, 1:NT], wp_s[0:M, 1:NT, M : M + 1], 0.0
            )

            # PE clock warm-up on zeroed scratch, runs during the x0 DMA wait
            nc.vector.memset(scratch[:], 0.0)
            ps_warm = pspool.tile([P, NFREE // 2], F32, tag="pslo")
            for _ in range(3):
                nc.tensor.matmul(
                    ps_warm[:, 0:512], scratch[:, 0:P], scratch[:], start=True, stop=True
                )

            for t in range(NT):
                ps_lo = pspool.tile([P, half], F32, tag="pslo")
                ps_hi = pspool.tile([P, half], F32, tag="pshi")
                for F in range(NB):
                    s = slice(F * 512, (F + 1) * 512)
                    dst = ps_lo if F < 2 else ps_hi
                    ds = slice((F % 2) * 512, (F % 2) * 512 + 512)
                    nc.tensor.matmul(
                        dst[0:M, ds],
                        wp_s[:, t, 0:M],
                        x_s[:, t, s],
                        start=True,
                        stop=True,
                    )
                nc.scalar.activation(
                    out_s[:, t, 0:half],
                    ps_lo[0:M],
                    mybir.ActivationFunctionType.Identity,
                    bias=bias_s[:, t : t + 1],
                )
                nc.vector.tensor_scalar_add(
                    out_s[:, t, half:NFREE],
                    ps_hi[0:M],
                    bias_s[:, t : t + 1],
                )
                nc.gpsimd.dma_start(out_d[:, t], out_s[:, t])

    nc.compile()
    return nc


_NC = None


def _get_nc():
    global _NC
    if _NC is None:
        _NC = _build_nc()
    return _NC


def _make_in_maps(x, W, b):
    wps = _host_weights(
        np.asarray(W, dtype=np.float32), np.asarray(b, dtype=np.float32)
    )
    xh = _host_x(np.asarray(x, dtype=np.float32))
    return [{"x": xh[c], "wp": wps[c]} for c in range(NCORES)]


def _gather(results):
    out = np.empty((B, L, C), np.float32)
    for c, r in enumerate(results):
        oh = np.asarray(r["out"]).reshape(M, NT, B, C)  # [104, 5, B, C]
        for t in range(NT):
            mt = min(M, ROWS_PC - t * M)
            r0 = c * ROWS_PC + t * M
            out[:, r0 : r0 + mt] = oh[:mt, t].transpose(1, 0, 2)
    return out


def kernel(x: np.ndarray, W: np.ndarray, b: np.ndarray) -> np.ndarray:
    nc = _get_nc()
    res = run_bass_kernel_spmd(nc, _make_in_maps(x, W, b), list(range(NCORES)))
    return _gather(res.results)


if __name__ == "__main__":
    rng = np.random.default_rng(0)
    x = rng.standard_normal((B, L, C), dtype=np.float32)
    W = rng.standard_normal((L, PADDED), dtype=np.float32) * 0.02
    b = rng.standard_normal((L,), dtype=np.float32) * 0.02
    print(kernel(x, W, b).shape)
